# Initial kernel scaffold
#
import sys
import math

if "/opt/trn_rl_repo" not in sys.path:
    sys.path.insert(0, "/opt/trn_rl_repo")

import numpy as np
from contextlib import ExitStack

import concourse.bass as bass
import concourse.bacc as bacc
import concourse.mybir as mybir
import concourse.tile as tile
from concourse.bass_utils import run_bass_kernel_spmd

F32 = mybir.dt.float32
F32R = mybir.dt.float32r
EXP = mybir.ActivationFunctionType.Exp
MULT = mybir.AluOpType.mult
ADD = mybir.AluOpType.add
AXX = mybir.AxisListType.X

B, H, L, D, M = 8, 4, 4096, 128, 640
MP = 768
NCORES = 8
NBH = (B * H) // NCORES
NEG_GSCALE = -1.0 / (2.0 * math.sqrt(D))


def r(ap):
    return ap.bitcast(F32R)


def build_bass(n_bh=NBH, seq=L):
    nc = bacc.Bacc("TRN2", debug=False)
    q = nc.dram_tensor("q", [n_bh, seq, D], F32, kind="ExternalInput").ap()
    k = nc.dram_tensor("k", [n_bh, seq, D], F32, kind="ExternalInput").ap()
    v = nc.dram_tensor("v", [n_bh, seq, D], F32, kind="ExternalInput").ap()
    projT = nc.dram_tensor("projT", [D, MP], F32, kind="ExternalInput").ap()
    ident = nc.dram_tensor("ident", [128, 128], F32, kind="ExternalInput").ap()
    out = nc.dram_tensor("out", [n_bh, seq, D], F32, kind="ExternalOutput").ap()

    assert seq % 512 == 0
    ngrp = seq // 512
    ntile = 4 * ngrp

    def ldma(sbuf_tile, dram_ap, g):
        nc.sync.dma_start(
            sbuf_tile[:],
            dram_ap[512 * g : 512 * (g + 1), :].rearrange("(t p) d -> p t d", t=4, p=128),
        )

    with tile.TileContext(nc) as tc, ExitStack() as ctx:
        const = ctx.enter_context(tc.tile_pool(name="const", bufs=1))
        projT_sb = const.tile([D, MP], F32)
        nc.sync.dma_start(projT_sb[:], projT)
        ident_sb = const.tile([128, 128], F32)
        nc.sync.dma_start(ident_sb[:], ident)
        ones_f = const.tile([1, 128], F32)
        nc.vector.memset(ones_f[:], 1.0)
        ones_sb = const.tile([1, 128], F32R)
        nc.vector.tensor_copy(ones_sb[:], ones_f[:])
        projT_r = const.tile([D, MP], F32R)
        nc.vector.tensor_copy(projT_r[:], projT_sb[:])

        ld_k = ctx.enter_context(tc.tile_pool(name="ld_k", bufs=3))
        ld_v = ctx.enter_context(tc.tile_pool(name="ld_v", bufs=3))
        ld_q = ctx.enter_context(tc.tile_pool(name="ld_q", bufs=3))
        kt_p = ctx.enter_context(tc.tile_pool(name="kt_sb", bufs=2))
        qt_p = ctx.enter_context(tc.tile_pool(name="qt_sb", bufs=2))
        phik_p = ctx.enter_context(tc.tile_pool(name="phik", bufs=3))
        phiq_p = ctx.enter_context(tc.tile_pool(name="phiq", bufs=2))
        misc_p = ctx.enter_context(tc.tile_pool(name="misc", bufs=3))
        acc_p = ctx.enter_context(tc.tile_pool(name="acc", bufs=2))
        ctxsb_p = ctx.enter_context(tc.tile_pool(name="ctxsb", bufs=2))
        num_p = ctx.enter_context(tc.tile_pool(name="numsb", bufs=2))
        outsb_p = ctx.enter_context(tc.tile_pool(name="outsb", bufs=2))

        for bh in range(n_bh):
            acc_d = acc_p.tile([128, M], F32, tag="acc_d")
            acc_g = acc_p.tile([128, M], F32, tag="acc_g")
            ctxT_sb = ctxsb_p.tile([128, M], F32, tag="ctxT")
            with tc.tile_pool(name="ps_kt", bufs=2, space="PSUM") as ps_kt, \
                 tc.tile_pool(name="ps_arr", bufs=2, space="PSUM") as ps_arr, \
                 tc.tile_pool(name="ps_ctx", bufs=1, space="PSUM") as ps_ctx:
                ctxT_ps = ps_ctx.tile([128, 1024], F32)
                for g in range(ngrp):
                    k_buf = ld_k.tile([128, 4, D], F32, tag="k")
                    ldma(k_buf, k[bh], g)
                    v_buf = ld_v.tile([128, 4, D], F32, tag="v")
                    ldma(v_buf, v[bh], g)
                    vr = ld_v.tile([128, 4, D], F32R, tag="vr")
                    nc.vector.tensor_copy(vr[:], v_buf[:])
                    negb = misc_p.tile([128, 4], F32, tag="negb")
                    gscr = misc_p.tile([128, 4, D], F32, tag="gscr")
                    nc.vector.tensor_mul(gscr[:], k_buf[:], k_buf[:])
                    nc.vector.reduce_sum(negb[:], gscr[:], axis=AXX)
                    nc.vector.tensor_scalar_mul(negb[:], negb[:], NEG_GSCALE)
                    kt_ps = ps_kt.tile([128, 512], F32, tag="kt")
                    for t in range(4):
                        nc.tensor.transpose(
                            kt_ps[:, 128 * t : 128 * (t + 1)],
                            k_buf[:, t, :],
                            ident_sb[:],
                        )
                    kt_sb = kt_p.tile([128, 512], F32R, tag="kt")
                    nc.vector.tensor_copy(kt_sb[:], kt_ps[:])
                    for t in range(4):
                        gi = 4 * g + t
                        arr = ps_arr.tile([128, 1024], F32, tag="arr")
                        lhsT = kt_sb[:, 128 * t : 128 * (t + 1)]
                        nc.tensor.matmul(arr[:, 0:512], lhsT, projT_r[:, 0:512])
                        nc.tensor.matmul(arr[:, 512:768], lhsT, projT_r[:, 512:768])
                        phik = phik_p.tile([128, MP], F32R, tag="phik")
                        nc.scalar.activation(
                            phik[:], arr[:, 0:MP], EXP, bias=negb[:, t : t + 1], scale=1.0
                        )
                        first = gi == 0
                        last = gi == ntile - 1
                        nc.tensor.matmul(
                            ctxT_ps[:, 0:512], vr[:, t, :], phik[:, 0:512],
                            start=first, stop=last,
                        )
                        nc.tensor.matmul(
                            ctxT_ps[:, 512:768], vr[:, t, :], phik[:, 512:768],
                            start=first, stop=last,
                        )
                        phikf = phik[:, 0:M].bitcast(F32)
                        if gi == 0:
                            nc.vector.tensor_copy(acc_d[:], phikf)
                        elif gi == 1:
                            nc.gpsimd.tensor_copy(acc_g[:], phikf)
                        elif gi % 2 == 0:
                            nc.vector.tensor_add(acc_d[:], acc_d[:], phikf)
                        else:
                            nc.gpsimd.tensor_add(acc_g[:], acc_g[:], phikf)
                nc.vector.tensor_copy(ctxT_sb[:], ctxT_ps[:, 0:M])
                nc.vector.tensor_add(acc_d[:], acc_d[:], acc_g[:])

            ctx_sb = ctxsb_p.tile([128, M], F32R, tag="ctx")
            ksum5 = ctxsb_p.tile([128, 8], F32R, tag="ksum5")
            with tc.tile_pool(name="ps_epi", bufs=2, space="PSUM") as ps_epi:
                fixT = ps_epi.tile([128, 1024], F32, tag="fix")
                for j in range(5):
                    nc.tensor.transpose(
                        fixT[:, 128 * j : 128 * (j + 1)],
                        ctxT_sb[:, 128 * j : 128 * (j + 1)],
                        ident_sb[:],
                    )
                nc.vector.tensor_copy(ctx_sb[:], fixT[:, 0:M])
                ksT = ps_epi.tile([128, 1024], F32, tag="fix")
                for j in range(5):
                    nc.tensor.transpose(
                        ksT[:, 128 * j : 128 * (j + 1)],
                        acc_d[:, 128 * j : 128 * (j + 1)],
                        ident_sb[:],
                    )
                with nc.allow_low_precision(reason="fp32r rounding for PE consumption"):
                    nc.vector.reduce_sum(
                        ksum5[:, 0:4],
                        ksT[:, 0:512].rearrange("p (j x) -> p j x", j=4),
                        axis=AXX,
                    )
                    nc.vector.reduce_sum(ksum5[:, 4:5], ksT[:, 512:640], axis=AXX)

            with tc.tile_pool(name="ps_qt", bufs=1, space="PSUM") as ps_qt, \
                 tc.tile_pool(name="ps_arrq", bufs=2, space="PSUM") as ps_arrq, \
                 tc.tile_pool(name="ps_nd", bufs=1, space="PSUM") as ps_nd:
                for g in range(ngrp):
                    q_buf = ld_q.tile([128, 4, D], F32, tag="q")
                    ldma(q_buf, q[bh], g)
                    out_sb = outsb_p.tile([128, 4, D], F32, tag="out")
                    for hh in range(2):
                        qt_ps = ps_qt.tile([128, 256], F32, tag="qt")
                        for t in range(2):
                            nc.tensor.transpose(
                                qt_ps[:, 128 * t : 128 * (t + 1)],
                                q_buf[:, 2 * hh + t, :],
                                ident_sb[:],
                            )
                        qt_sb = qt_p.tile([128, 256], F32R, tag="qt")
                        nc.vector.tensor_copy(qt_sb[:], qt_ps[:])
                        arrq = ps_arrq.tile([128, 1536], F32, tag="arrq")
                        for j in range(5):
                            nc.tensor.matmul(
                                arrq[:, 256 * j : 256 * (j + 1)],
                                projT_r[:, 128 * j : 128 * (j + 1)],
                                qt_sb[:],
                            )
                        phiq = phiq_p.tile([128, 1280], F32R, tag="phiq")
                        nc.scalar.activation(phiq[:], arrq[:, 0:1280], EXP, bias=0.0, scale=1.0)
                        nd = ps_nd.tile([128, 512], F32, tag="nd")
                        for j in range(5):
                            nc.tensor.matmul(
                                nd[:, 0:256],
                                ctx_sb[:, 128 * j : 128 * (j + 1)],
                                phiq[:, 256 * j : 256 * (j + 1)],
                                start=(j == 0), stop=(j == 4),
                            )
                        for j in range(5):
                            nc.tensor.matmul(
                                nd[0:1, 256:512],
                                ksum5[:, j : j + 1],
                                phiq[:, 256 * j : 256 * (j + 1)],
                                start=(j == 0), stop=(j == 4),
                            )
                        recip_row = misc_p.tile([1, 256], F32R, tag="recip")
                        with nc.allow_low_precision(reason="fp32r rounding for PE consumption"):
                            nc.vector.reciprocal(recip_row[:], nd[0:1, 256:512])
                        nc.tensor.matmul(nd[:, 256:512], ones_sb[:], recip_row[:])
                        rb_sb = num_p.tile([128, 256], F32, tag="rb")
                        nc.vector.tensor_copy(rb_sb[:], nd[:, 256:512])
                        numn = num_p.tile([128, 256], F32, tag="numn")
                        nc.vector.tensor_mul(numn[:], nd[:, 0:256], rb_sb[:])
                        outT = ps_nd.tile([128, 512], F32, tag="nd")
                        for t in range(2):
                            nc.tensor.transpose(
                                outT[:, 128 * t : 128 * (t + 1)],
                                numn[:, 128 * t : 128 * (t + 1)],
                                ident_sb[:],
                            )
                        nc.vector.tensor_copy(out_sb[:, 2 * hh : 2 * hh + 2, :], outT[:, 0:256])
                    nc.sync.dma_start(
                        out[bh, 512 * g : 512 * (g + 1), :].rearrange(
                            "(t p) d -> p t d", t=4, p=128
                        ),
                        out_sb[:],
                    )
    nc.compile()
    return nc


_NC_CACHE = {}


def _get_nc(n_bh=NBH, seq=L):
    key = (n_bh, seq)
    if key not in _NC_CACHE:
        _NC_CACHE[key] = build_bass(n_bh, seq)
    return _NC_CACHE[key]


def host_inputs(projection_matrix):
    projT_pad = np.zeros((D, MP), dtype=np.float32)
    projT_pad[:, :M] = np.ascontiguousarray(
        (np.asarray(projection_matrix, dtype=np.float32) / (D**0.25)).T
    )
    ident = np.eye(128, dtype=np.float32)
    return projT_pad, ident


def kernel(q, k, v, projection_matrix, _trace=False, _trace_kwargs=None):
    q = np.ascontiguousarray(np.asarray(q, dtype=np.float32)).reshape(B * H, L, D)
    k = np.ascontiguousarray(np.asarray(k, dtype=np.float32)).reshape(B * H, L, D)
    v = np.ascontiguousarray(np.asarray(v, dtype=np.float32)).reshape(B * H, L, D)
    projT_pad, ident = host_inputs(projection_matrix)

    in_maps = []
    for c in range(NCORES):
        sl = slice(NBH * c, NBH * (c + 1))
        in_maps.append(
            {
                "q": np.ascontiguousarray(q[sl]),
                "k": np.ascontiguousarray(k[sl]),
                "v": np.ascontiguousarray(v[sl]),
                "projT": projT_pad,
                "ident": ident,
            }
        )

    nc = _get_nc()
    kwargs = {}
    if _trace:
        kwargs["trace"] = True
        kwargs.update(_trace_kwargs or {})
    res = run_bass_kernel_spmd(nc, in_maps, core_ids=list(range(NCORES)), **kwargs)
    outs = np.concatenate([res.results[c]["out"] for c in range(NCORES)], axis=0)
    result = outs.reshape(B, H, L, D).astype(np.float32)
    if _trace:
        return result, res
    return result


def timed_run(q, k, v, projection_matrix, iters=5):
    import time
    import jax
    from jax.sharding import Mesh, PartitionSpec
    from jax.experimental.shard_map import shard_map
    from concourse import bass2jax

    q = np.ascontiguousarray(np.asarray(q, dtype=np.float32)).reshape(B * H, L, D)
    k = np.ascontiguousarray(np.asarray(k, dtype=np.float32)).reshape(B * H, L, D)
    v = np.ascontiguousarray(np.asarray(v, dtype=np.float32)).reshape(B * H, L, D)
    projT_pad, ident = host_inputs(projection_matrix)
    nc = _get_nc()
    bass2jax.install_neuronx_cc_hook()

    in_names = []
    out_names = []
    out_avals = []
    zero_outs = []
    import concourse.mybir as mybir_

    partition_name = nc.partition_id_tensor.name if nc.partition_id_tensor else None
    for alloc in nc.m.functions[0].allocations:
        if not isinstance(alloc, mybir_.MemoryLocationSet):
            continue
        name = alloc.memorylocations[0].name
        if alloc.kind == "ExternalInput":
            if name != partition_name:
                in_names.append(name)
        elif alloc.kind == "ExternalOutput":
            out_names.append(name)
            shape = list(alloc.tensor_shape)
            out_avals.append(jax.core.ShapedArray(shape, np.float32))
            zero_outs.append(np.zeros(shape, np.float32))
    n_params = len(in_names)
    n_outs = len(out_names)
    all_names = in_names + out_names
    if partition_name is not None:
        all_names = all_names + [partition_name]

    def _body(*args):
        operands = list(args)
        if partition_name is not None:
            operands.append(bass2jax.partition_id_tensor())
        outs = bass2jax._bass_exec_p.bind(
            *operands,
            out_avals=tuple(out_avals),
            in_names=tuple(all_names),
            out_names=tuple(out_names),
            lowering_input_output_aliases=(),
            sim_require_finite=True,
            sim_require_nnan=True,
            nc=nc,
        )
        return tuple(outs)

    devices = jax.devices()[:NCORES]
    mesh = Mesh(np.asarray(devices), ("core",))
    in_specs = (PartitionSpec("core"),) * (n_params + n_outs)
    out_specs = (PartitionSpec("core"),) * n_outs
    sharded = jax.jit(
        shard_map(_body, mesh=mesh, in_specs=in_specs, out_specs=out_specs, check_rep=False),
        keep_unused=True,
    )

    per_core_vals = {
        "q": [q[NBH * c : NBH * (c + 1)] for c in range(NCORES)],
        "k": [k[NBH * c : NBH * (c + 1)] for c in range(NCORES)],
        "v": [v[NBH * c : NBH * (c + 1)] for c in range(NCORES)],
        "projT": [projT_pad] * NCORES,
        "ident": [ident] * NCORES,
    }
    concat_in = [
        np.concatenate(per_core_vals[nm], axis=0) for nm in in_names
    ]
    concat_zeros = [
        np.zeros((NCORES * z.shape[0], *z.shape[1:]), z.dtype) for z in zero_outs
    ]
    sharding = jax.sharding.NamedSharding(mesh, PartitionSpec("core"))
    dev_in = [jax.device_put(a, sharding) for a in concat_in]
    dev_zero = [jax.device_put(a, sharding) for a in concat_zeros]
    r0 = sharded(*dev_in, *dev_zero)
    jax.block_until_ready(r0)
    times = []
    for _ in range(iters):
        t0 = time.perf_counter()
        rr = sharded(*dev_in, *dev_zero)
        jax.block_until_ready(rr)
        times.append(time.perf_counter() - t0)
    out = np.asarray(rr[out_names.index("out")]).reshape(NCORES, NBH, L, D)
    result = out.reshape(B, H, L, D)
    return result, times



# revision 1
# speedup vs baseline: 1.1026x; 1.1026x over previous
import sys
import math

if "/opt/trn_rl_repo" not in sys.path:
    sys.path.insert(0, "/opt/trn_rl_repo")

import numpy as np
from contextlib import ExitStack

import concourse.bass as bass
import concourse.bacc as bacc
import concourse.mybir as mybir
import concourse.tile as tile
from concourse.bass_utils import run_bass_kernel_spmd

F32 = mybir.dt.float32
F32R = mybir.dt.float32r
EXP = mybir.ActivationFunctionType.Exp
MULT = mybir.AluOpType.mult
ADD = mybir.AluOpType.add
AXX = mybir.AxisListType.X

B, H, L, D, M = 8, 4, 4096, 128, 640
MP = 768
NCORES = 8
NBH = (B * H) // NCORES
NEG_GSCALE = -1.0 / (2.0 * math.sqrt(D))


def r(ap):
    return ap.bitcast(F32R)


def build_bass(n_bh=NBH, seq=L):
    nc = bacc.Bacc("TRN2", debug=False)
    q = nc.dram_tensor("q", [n_bh, seq, D], F32, kind="ExternalInput").ap()
    k = nc.dram_tensor("k", [n_bh, seq, D], F32, kind="ExternalInput").ap()
    v = nc.dram_tensor("v", [n_bh, seq, D], F32, kind="ExternalInput").ap()
    projT = nc.dram_tensor("projT", [D, MP], F32, kind="ExternalInput").ap()
    ident = nc.dram_tensor("ident", [128, 128], F32, kind="ExternalInput").ap()
    out = nc.dram_tensor("out", [n_bh, seq, D], F32, kind="ExternalOutput").ap()

    assert seq % 512 == 0
    ngrp = seq // 512
    ntile = 4 * ngrp

    def ldma(sbuf_tile, dram_ap, g):
        nc.sync.dma_start(
            sbuf_tile[:],
            dram_ap[512 * g : 512 * (g + 1), :].rearrange("(t p) d -> p t d", t=4, p=128),
        )

    with tile.TileContext(nc) as tc, ExitStack() as ctx:
        const = ctx.enter_context(tc.tile_pool(name="const", bufs=1))
        projT_sb = const.tile([D, MP], F32)
        nc.sync.dma_start(projT_sb[:], projT)
        ident_sb = const.tile([128, 128], F32)
        nc.sync.dma_start(ident_sb[:], ident)
        ones_f = const.tile([1, 128], F32)
        nc.vector.memset(ones_f[:], 1.0)
        ones_sb = const.tile([1, 128], F32R)
        nc.vector.tensor_copy(ones_sb[:], ones_f[:])
        projT_r = const.tile([D, MP], F32R)
        nc.vector.tensor_copy(projT_r[:], projT_sb[:])

        ld_k = ctx.enter_context(tc.tile_pool(name="ld_k", bufs=3))
        ld_v = ctx.enter_context(tc.tile_pool(name="ld_v", bufs=3))
        ld_q = ctx.enter_context(tc.tile_pool(name="ld_q", bufs=3))
        kt_p = ctx.enter_context(tc.tile_pool(name="kt_sb", bufs=2))
        qt_p = ctx.enter_context(tc.tile_pool(name="qt_sb", bufs=2))
        phik_p = ctx.enter_context(tc.tile_pool(name="phik", bufs=3))
        phiq_p = ctx.enter_context(tc.tile_pool(name="phiq", bufs=2))
        misc_p = ctx.enter_context(tc.tile_pool(name="misc", bufs=3))
        acc_p = ctx.enter_context(tc.tile_pool(name="acc", bufs=2))
        ctxsb_p = ctx.enter_context(tc.tile_pool(name="ctxsb", bufs=2))
        num_p = ctx.enter_context(tc.tile_pool(name="numsb", bufs=2))
        outsb_p = ctx.enter_context(tc.tile_pool(name="outsb", bufs=2))

        for bh in range(n_bh):
            acc_d = acc_p.tile([128, M], F32, tag="acc_d")
            acc_g = acc_p.tile([128, M], F32, tag="acc_g")
            ctxT_sb = ctxsb_p.tile([128, M], F32, tag="ctxT")
            with tc.tile_pool(name="ps_kt", bufs=2, space="PSUM") as ps_kt, \
                 tc.tile_pool(name="ps_arr", bufs=2, space="PSUM") as ps_arr, \
                 tc.tile_pool(name="ps_ctx", bufs=1, space="PSUM") as ps_ctx:
                ctxT_ps = ps_ctx.tile([128, 1024], F32)
                for g in range(ngrp):
                    k_buf = ld_k.tile([128, 4, D], F32, tag="k")
                    ldma(k_buf, k[bh], g)
                    v_buf = ld_v.tile([128, 4, D], F32, tag="v")
                    ldma(v_buf, v[bh], g)
                    vr = ld_v.tile([128, 4, D], F32R, tag="vr")
                    nc.vector.tensor_copy(vr[:], v_buf[:])
                    negb = misc_p.tile([128, 4], F32, tag="negb")
                    gscr = misc_p.tile([128, 4, D], F32, tag="gscr")
                    nc.vector.tensor_mul(gscr[:], k_buf[:], k_buf[:])
                    nc.vector.reduce_sum(negb[:], gscr[:], axis=AXX)
                    nc.vector.tensor_scalar_mul(negb[:], negb[:], NEG_GSCALE)
                    kt_ps = ps_kt.tile([128, 512], F32, tag="kt")
                    for t in range(4):
                        nc.tensor.transpose(
                            kt_ps[:, 128 * t : 128 * (t + 1)],
                            k_buf[:, t, :],
                            ident_sb[:],
                        )
                    kt_sb = kt_p.tile([128, 512], F32R, tag="kt")
                    nc.vector.tensor_copy(kt_sb[:], kt_ps[:])
                    for t in range(4):
                        gi = 4 * g + t
                        arr = ps_arr.tile([128, 1024], F32, tag="arr")
                        lhsT = kt_sb[:, 128 * t : 128 * (t + 1)]
                        nc.tensor.matmul(arr[:, 0:512], lhsT, projT_r[:, 0:512])
                        nc.tensor.matmul(arr[:, 512:768], lhsT, projT_r[:, 512:768])
                        phik = phik_p.tile([128, MP], F32R, tag="phik")
                        nc.scalar.activation(
                            phik[:], arr[:, 0:MP], EXP, bias=negb[:, t : t + 1], scale=1.0
                        )
                        first = gi == 0
                        last = gi == ntile - 1
                        nc.tensor.matmul(
                            ctxT_ps[:, 0:512], vr[:, t, :], phik[:, 0:512],
                            start=first, stop=last,
                        )
                        nc.tensor.matmul(
                            ctxT_ps[:, 512:768], vr[:, t, :], phik[:, 512:768],
                            start=first, stop=last,
                        )
                        phikf = phik[:, 0:M].bitcast(F32)
                        if gi == 0:
                            nc.vector.tensor_copy(acc_d[:], phikf)
                        elif gi == 1:
                            nc.gpsimd.tensor_copy(acc_g[:], phikf)
                        elif gi % 2 == 0:
                            nc.vector.tensor_add(acc_d[:], acc_d[:], phikf)
                        else:
                            nc.gpsimd.tensor_add(acc_g[:], acc_g[:], phikf)
                nc.vector.tensor_copy(ctxT_sb[:], ctxT_ps[:, 0:M])
                nc.vector.tensor_add(acc_d[:], acc_d[:], acc_g[:])

            ctx_sb = ctxsb_p.tile([128, M], F32R, tag="ctx")
            ksum5 = ctxsb_p.tile([128, 8], F32R, tag="ksum5")
            with tc.tile_pool(name="ps_epi", bufs=2, space="PSUM") as ps_epi:
                fixT = ps_epi.tile([128, 1024], F32, tag="fix")
                for j in range(5):
                    nc.tensor.transpose(
                        fixT[:, 128 * j : 128 * (j + 1)],
                        ctxT_sb[:, 128 * j : 128 * (j + 1)],
                        ident_sb[:],
                    )
                nc.vector.tensor_copy(ctx_sb[:], fixT[:, 0:M])
                ksT = ps_epi.tile([128, 1024], F32, tag="fix")
                for j in range(5):
                    nc.tensor.transpose(
                        ksT[:, 128 * j : 128 * (j + 1)],
                        acc_d[:, 128 * j : 128 * (j + 1)],
                        ident_sb[:],
                    )
                with nc.allow_low_precision(reason="fp32r rounding for PE consumption"):
                    nc.vector.reduce_sum(
                        ksum5[:, 0:4],
                        ksT[:, 0:512].rearrange("p (j x) -> p j x", j=4),
                        axis=AXX,
                    )
                    nc.vector.reduce_sum(ksum5[:, 4:5], ksT[:, 512:640], axis=AXX)

            with tc.tile_pool(name="ps_qt", bufs=1, space="PSUM") as ps_qt, \
                 tc.tile_pool(name="ps_arrq", bufs=2, space="PSUM") as ps_arrq, \
                 tc.tile_pool(name="ps_nd", bufs=1, space="PSUM") as ps_nd:
                for g in range(ngrp):
                    q_buf = ld_q.tile([128, 4, D], F32, tag="q")
                    ldma(q_buf, q[bh], g)
                    out_sb = outsb_p.tile([128, 4, D], F32, tag="out")
                    for hh in range(2):
                        qt_ps = ps_qt.tile([128, 256], F32, tag="qt")
                        for t in range(2):
                            nc.tensor.transpose(
                                qt_ps[:, 128 * t : 128 * (t + 1)],
                                q_buf[:, 2 * hh + t, :],
                                ident_sb[:],
                            )
                        qt_sb = qt_p.tile([128, 256], F32R, tag="qt")
                        nc.vector.tensor_copy(qt_sb[:], qt_ps[:])
                        arrq = ps_arrq.tile([128, 1536], F32, tag="arrq")
                        for j in range(5):
                            nc.tensor.matmul(
                                arrq[:, 256 * j : 256 * (j + 1)],
                                projT_r[:, 128 * j : 128 * (j + 1)],
                                qt_sb[:],
                            )
                        phiq = phiq_p.tile([128, 1280], F32R, tag="phiq")
                        nc.scalar.activation(phiq[:], arrq[:, 0:1280], EXP, bias=0.0, scale=1.0)
                        nd = ps_nd.tile([128, 512], F32, tag="nd")
                        for j in range(5):
                            nc.tensor.matmul(
                                nd[:, 0:256],
                                ctx_sb[:, 128 * j : 128 * (j + 1)],
                                phiq[:, 256 * j : 256 * (j + 1)],
                                start=(j == 0), stop=(j == 4),
                            )
                        for j in range(5):
                            nc.tensor.matmul(
                                nd[0:1, 256:512],
                                ksum5[:, j : j + 1],
                                phiq[:, 256 * j : 256 * (j + 1)],
                                start=(j == 0), stop=(j == 4),
                            )
                        recip_row = misc_p.tile([1, 256], F32R, tag="recip")
                        with nc.allow_low_precision(reason="fp32r rounding for PE consumption"):
                            nc.vector.reciprocal(recip_row[:], nd[0:1, 256:512])
                        nc.tensor.matmul(nd[:, 256:512], ones_sb[:], recip_row[:])
                        rb_sb = num_p.tile([128, 256], F32, tag="rb")
                        nc.vector.tensor_copy(rb_sb[:], nd[:, 256:512])
                        numn = num_p.tile([128, 256], F32, tag="numn")
                        nc.vector.tensor_mul(numn[:], nd[:, 0:256], rb_sb[:])
                        outT = ps_nd.tile([128, 512], F32, tag="nd")
                        for t in range(2):
                            nc.tensor.transpose(
                                outT[:, 128 * t : 128 * (t + 1)],
                                numn[:, 128 * t : 128 * (t + 1)],
                                ident_sb[:],
                            )
                        nc.vector.tensor_copy(out_sb[:, 2 * hh : 2 * hh + 2, :], outT[:, 0:256])
                    nc.sync.dma_start(
                        out[bh, 512 * g : 512 * (g + 1), :].rearrange(
                            "(t p) d -> p t d", t=4, p=128
                        ),
                        out_sb[:],
                    )
    nc.compile()
    return nc


_NC_CACHE = {}


def _get_nc(n_bh=NBH, seq=L):
    key = (n_bh, seq)
    if key not in _NC_CACHE:
        _NC_CACHE[key] = build_bass(n_bh, seq)
    return _NC_CACHE[key]


def host_inputs(projection_matrix):
    projT_pad = np.zeros((D, MP), dtype=np.float32)
    projT_pad[:, :M] = np.ascontiguousarray(
        (np.asarray(projection_matrix, dtype=np.float32) / (D**0.25)).T
    )
    ident = np.eye(128, dtype=np.float32)
    return projT_pad, ident


def kernel(q, k, v, projection_matrix, _trace=False, _trace_kwargs=None):
    q = np.ascontiguousarray(np.asarray(q, dtype=np.float32)).reshape(B * H, L, D)
    k = np.ascontiguousarray(np.asarray(k, dtype=np.float32)).reshape(B * H, L, D)
    v = np.ascontiguousarray(np.asarray(v, dtype=np.float32)).reshape(B * H, L, D)
    projT_pad, ident = host_inputs(projection_matrix)

    in_maps = []
    for c in range(NCORES):
        sl = slice(NBH * c, NBH * (c + 1))
        in_maps.append(
            {
                "q": np.ascontiguousarray(q[sl]),
                "k": np.ascontiguousarray(k[sl]),
                "v": np.ascontiguousarray(v[sl]),
                "projT": projT_pad,
                "ident": ident,
            }
        )

    nc = _get_nc()
    kwargs = {}
    if _trace:
        kwargs["trace"] = True
        kwargs.update(_trace_kwargs or {})
    res = run_bass_kernel_spmd(nc, in_maps, core_ids=list(range(NCORES)), **kwargs)
    outs = np.concatenate([res.results[c]["out"] for c in range(NCORES)], axis=0)
    result = outs.reshape(B, H, L, D).astype(np.float32)
    if _trace:
        return result, res
    return result


def timed_run(q, k, v, projection_matrix, iters=5):
    import time
    import jax
    from jax.sharding import Mesh, PartitionSpec
    from jax.experimental.shard_map import shard_map
    from concourse import bass2jax

    q = np.ascontiguousarray(np.asarray(q, dtype=np.float32)).reshape(B * H, L, D)
    k = np.ascontiguousarray(np.asarray(k, dtype=np.float32)).reshape(B * H, L, D)
    v = np.ascontiguousarray(np.asarray(v, dtype=np.float32)).reshape(B * H, L, D)
    projT_pad, ident = host_inputs(projection_matrix)
    nc = _get_nc()
    bass2jax.install_neuronx_cc_hook()

    in_names = []
    out_names = []
    out_avals = []
    zero_outs = []
    import concourse.mybir as mybir_

    partition_name = nc.partition_id_tensor.name if nc.partition_id_tensor else None
    for alloc in nc.m.functions[0].allocations:
        if not isinstance(alloc, mybir_.MemoryLocationSet):
            continue
        name = alloc.memorylocations[0].name
        if alloc.kind == "ExternalInput":
            if name != partition_name:
                in_names.append(name)
        elif alloc.kind == "ExternalOutput":
            out_names.append(name)
            shape = list(alloc.tensor_shape)
            out_avals.append(jax.core.ShapedArray(shape, np.float32))
            zero_outs.append(np.zeros(shape, np.float32))
    n_params = len(in_names)
    n_outs = len(out_names)
    all_names = in_names + out_names
    if partition_name is not None:
        all_names = all_names + [partition_name]

    def _body(*args):
        operands = list(args)
        if partition_name is not None:
            operands.append(bass2jax.partition_id_tensor())
        outs = bass2jax._bass_exec_p.bind(
            *operands,
            out_avals=tuple(out_avals),
            in_names=tuple(all_names),
            out_names=tuple(out_names),
            lowering_input_output_aliases=(),
            sim_require_finite=True,
            sim_require_nnan=True,
            nc=nc,
        )
        return tuple(outs)

    devices = jax.devices()[:NCORES]
    mesh = Mesh(np.asarray(devices), ("core",))
    in_specs = (PartitionSpec("core"),) * (n_params + n_outs)
    out_specs = (PartitionSpec("core"),) * n_outs
    sharded = jax.jit(
        shard_map(_body, mesh=mesh, in_specs=in_specs, out_specs=out_specs, check_rep=False),
        keep_unused=True,
    )

    per_core_vals = {
        "q": [q[NBH * c : NBH * (c + 1)] for c in range(NCORES)],
        "k": [k[NBH * c : NBH * (c + 1)] for c in range(NCORES)],
        "v": [v[NBH * c : NBH * (c + 1)] for c in range(NCORES)],
        "projT": [projT_pad] * NCORES,
        "ident": [ident] * NCORES,
    }
    concat_in = [
        np.concatenate(per_core_vals[nm], axis=0) for nm in in_names
    ]
    concat_zeros = [
        np.zeros((NCORES * z.shape[0], *z.shape[1:]), z.dtype) for z in zero_outs
    ]
    sharding = jax.sharding.NamedSharding(mesh, PartitionSpec("core"))
    dev_in = [jax.device_put(a, sharding) for a in concat_in]
    dev_zero = [jax.device_put(a, sharding) for a in concat_zeros]
    r0 = sharded(*dev_in, *dev_zero)
    jax.block_until_ready(r0)
    times = []
    for _ in range(iters):
        t0 = time.perf_counter()
        rr = sharded(*dev_in, *dev_zero)
        jax.block_until_ready(rr)
        times.append(time.perf_counter() - t0)
    out = np.asarray(rr[out_names.index("out")]).reshape(NCORES, NBH, L, D)
    result = out.reshape(B, H, L, D)
    return result, times



# revision 2
# speedup vs baseline: 395.5245x; 358.7347x over previous
import sys
import math

if "/opt/trn_rl_repo" not in sys.path:
    sys.path.insert(0, "/opt/trn_rl_repo")

import numpy as np
from contextlib import ExitStack

import concourse.bass as bass
import concourse.bacc as bacc
import concourse.mybir as mybir
import concourse.tile as tile
from concourse.bass_utils import run_bass_kernel_spmd

F32 = mybir.dt.float32
F16 = mybir.dt.float16
BF16 = mybir.dt.bfloat16
EXP = mybir.ActivationFunctionType.Exp
MULT = mybir.AluOpType.mult
ADD = mybir.AluOpType.add
AXX = mybir.AxisListType.X

B, H, L, D, M = 8, 4, 4096, 128, 640
NCORES = 8
NBH = (B * H) // NCORES
NEG_GSCALE = -1.0 / (2.0 * math.sqrt(D))


def build_bass(n_bh=NBH, seq=L):
    nc = bacc.Bacc("TRN2", debug=False)
    q = nc.dram_tensor("q", [n_bh, seq, D], F16, kind="ExternalInput").ap()
    k = nc.dram_tensor("k", [n_bh, seq, D], F16, kind="ExternalInput").ap()
    v = nc.dram_tensor("v", [n_bh, seq, D], F16, kind="ExternalInput").ap()
    projT = nc.dram_tensor("projT", [D, M], F16, kind="ExternalInput").ap()
    ident = nc.dram_tensor("ident", [128, 128], F16, kind="ExternalInput").ap()
    out = nc.dram_tensor("out", [n_bh, seq, D], F32, kind="ExternalOutput").ap()

    assert seq % 1024 == 0
    nsg = seq // 1024
    ntile = 8 * nsg

    def ldma(sbuf_tile, dram_ap, sg):
        nc.sync.dma_start(
            sbuf_tile[:],
            dram_ap[1024 * sg : 1024 * (sg + 1), :].rearrange(
                "(t p) d -> p t d", t=8, p=128
            ),
        )

    with tile.TileContext(nc) as tc, ExitStack() as ctx:
        const = ctx.enter_context(tc.tile_pool(name="const", bufs=1))
        warm = const.tile([1, 2], F32)
        nc.vector.memset(warm[:, 0:1], 0.0)
        nc.scalar.activation(warm[:, 1:2], warm[:, 0:1], EXP, bias=0.0, scale=1.0)
        projT_sb = const.tile([D, M], F16)
        ident_sb = const.tile([128, 128], F16)

        ld_k = ctx.enter_context(tc.tile_pool(name="ld_k", bufs=2))
        ld_v = ctx.enter_context(tc.tile_pool(name="ld_v", bufs=2))
        ld_q = ctx.enter_context(tc.tile_pool(name="ld_q", bufs=2))
        kt_p = ctx.enter_context(tc.tile_pool(name="kt_sb", bufs=2))
        qt_p = ctx.enter_context(tc.tile_pool(name="qt_sb", bufs=2))
        phik_p = ctx.enter_context(tc.tile_pool(name="phik", bufs=3))
        phiq_p = ctx.enter_context(tc.tile_pool(name="phiq", bufs=13))
        misc_p = ctx.enter_context(tc.tile_pool(name="misc", bufs=2))
        acc_p = ctx.enter_context(tc.tile_pool(name="acc", bufs=2))
        ctxsb_p = ctx.enter_context(tc.tile_pool(name="ctxsb", bufs=2))
        outsb_p = ctx.enter_context(tc.tile_pool(name="outsb", bufs=2))
        ps_qt = ctx.enter_context(tc.tile_pool(name="ps_qt", bufs=1, space="PSUM"))

        def qt_head(q_buf, h):
            qt_ps = ps_qt.tile([128, 512], F16, tag="qt")
            for t in range(4):
                nc.tensor.transpose(
                    qt_ps[:, 128 * t : 128 * (t + 1)],
                    q_buf[:, 4 * h + t, :],
                    ident_sb[:],
                )
            qt_sb = qt_p.tile([128, 512], F16, tag="qt")
            nc.gpsimd.tensor_copy(qt_sb[:], qt_ps[:])
            return qt_sb

        for bh in range(n_bh):
            acc_a = acc_p.tile([128, M], F16, tag="acc_a")
            acc_b = acc_p.tile([128, M], F16, tag="acc_b")
            ctxT_sb = ctxsb_p.tile([128, M], F16, tag="ctxT")
            q_buf0 = None
            qt_sb0 = None
            with tc.tile_pool(name="ps_ctx", bufs=1, space="PSUM") as ps_ctx, \
                 tc.tile_pool(name="ps_arr", bufs=2, space="PSUM") as ps_arr, \
                 tc.tile_pool(name="ps_kt", bufs=1, space="PSUM") as ps_kt:
                ctxT_ps = ps_ctx.tile([128, M], F32)
                for sg in range(nsg):
                    k_buf = ld_k.tile([128, 8, D], F16, tag="k")
                    if bh == 0 and sg == 0:
                        nc.sync.dma_start(
                            k_buf[:, 0:4, :],
                            k[bh][0:512, :].rearrange("(t p) d -> p t d", t=4, p=128),
                        )
                        nc.sync.dma_start(ident_sb[:], ident)
                        nc.sync.dma_start(
                            k_buf[:, 4:8, :],
                            k[bh][512:1024, :].rearrange("(t p) d -> p t d", t=4, p=128),
                        )
                        nc.sync.dma_start(projT_sb[:], projT)
                    else:
                        ldma(k_buf, k[bh], sg)
                    v_buf = ld_v.tile([128, 8, D], F16, tag="v")
                    ldma(v_buf, v[bh], sg)
                    negb = misc_p.tile([128, 8], F32, tag="negb")
                    gscr = misc_p.tile([128, 8, D], F16, tag="gscr")
                    for u in range(8):
                        nc.vector.tensor_tensor_reduce(
                            out=gscr[:, u, :],
                            in0=k_buf[:, u, :],
                            in1=k_buf[:, u, :],
                            scale=NEG_GSCALE,
                            scalar=0.0,
                            op0=MULT,
                            op1=ADD,
                            accum_out=negb[:, u : u + 1],
                        )
                    for h in range(2):
                        kt_ps = ps_kt.tile([128, 512], F16, tag="kt")
                        for t in range(4):
                            nc.tensor.transpose(
                                kt_ps[:, 128 * t : 128 * (t + 1)],
                                k_buf[:, 4 * h + t, :],
                                ident_sb[:],
                            )
                        kt_sb = kt_p.tile([128, 512], F16, tag="kt")
                        nc.gpsimd.tensor_copy(kt_sb[:], kt_ps[:])
                        if sg == nsg - 1 and h == 1:
                            q_buf0 = ld_q.tile([128, 8, D], F16, tag="q")
                            ldma(q_buf0, q[bh], 0)
                            qt_sb0 = qt_head(q_buf0, 0)
                        for t in range(4):
                            u = 4 * h + t
                            gi = 8 * sg + u
                            arr = ps_arr.tile([128, M], F32, tag="arr")
                            lhsT = kt_sb[:, 128 * t : 128 * (t + 1)]
                            nc.tensor.matmul(arr[:, 0:512], lhsT, projT_sb[:, 0:512])
                            nc.tensor.matmul(arr[:, 512:M], lhsT, projT_sb[:, 512:M])
                            phik = phik_p.tile([128, M], F16, tag="phik")
                            nc.scalar.activation(
                                phik[:], arr[:], EXP, bias=negb[:, u : u + 1], scale=1.0
                            )
                            first = gi == 0
                            last = gi == ntile - 1
                            nc.tensor.matmul(
                                ctxT_ps[:, 0:512], v_buf[:, u, :], phik[:, 0:512],
                                start=first, stop=last,
                            )
                            nc.tensor.matmul(
                                ctxT_ps[:, 512:M], v_buf[:, u, :], phik[:, 512:M],
                                start=first, stop=last,
                            )
                            if gi == 0:
                                nc.vector.tensor_copy(acc_a[:], phik[:])
                            elif gi == 1:
                                nc.vector.tensor_copy(acc_b[:], phik[:])
                            elif gi % 2 == 0:
                                nc.vector.tensor_add(acc_a[:], acc_a[:], phik[:])
                            else:
                                nc.vector.tensor_add(acc_b[:], acc_b[:], phik[:])
                nc.vector.tensor_copy(ctxT_sb[:], ctxT_ps[:])
                nc.vector.tensor_add(acc_a[:], acc_a[:], acc_b[:])

            ctx_aug = ctxsb_p.tile([128, 5, 129], BF16, tag="ctx_aug")
            with tc.tile_pool(name="ps_nd", bufs=2, space="PSUM") as ps_nd, \
                 tc.tile_pool(name="ps_arrq", bufs=2, space="PSUM") as ps_arrq:

                def q_head_rest(qt_sb):
                    phiqs = []
                    for t in range(4):
                        arrq = ps_arrq.tile([128, 5, 128], F32, tag="arrq")
                        rhs = qt_sb[:, 128 * t : 128 * (t + 1)]
                        for j in range(5):
                            nc.tensor.matmul(
                                arrq[:, j, :],
                                projT_sb[:, 128 * j : 128 * (j + 1)],
                                rhs,
                            )
                        phiq = phiq_p.tile([128, 5, 128], BF16, tag="phiq")
                        nc.scalar.activation(phiq[:], arrq[:], EXP, bias=0.0, scale=1.0)
                        phiqs.append(phiq)
                    return phiqs

                def q_tail(phiqs, out_sb, h):
                    for t in range(4):
                        nd = ps_nd.tile([128, 129], F32, tag="nd")
                        for j in range(5):
                            nc.tensor.matmul(
                                nd[:],
                                phiqs[t][:, j, :],
                                ctx_aug[:, j, :],
                                start=(j == 0), stop=(j == 4),
                            )
                        recip = misc_p.tile([128, 1], F32, tag="recip")
                        nc.vector.reciprocal(recip[:], nd[:, 128:129])
                        nc.gpsimd.tensor_scalar_mul(
                            out_sb[:, 4 * h + t, :], nd[:, 0:128], recip[:]
                        )

                q_bufs = {0: q_buf0}
                out_sbs = {}
                pending = []

                def emit_head(b):
                    sg, h = divmod(b, 2)
                    if h == 0:
                        if sg not in q_bufs:
                            q_bufs[sg] = ld_q.tile([128, 8, D], F16, tag="q", name=f"q_buf{sg}")
                            ldma(q_bufs[sg], q[bh], sg)
                        out_sbs[sg] = outsb_p.tile([128, 8, D], F32, tag="out", name=f"out_sb{sg}")
                    qt_sb = qt_sb0 if b == 0 else qt_head(q_bufs[sg], h)
                    pending.append((q_head_rest(qt_sb), sg, h))

                def emit_tail():
                    phiqs, sg, h = pending.pop(0)
                    q_tail(phiqs, out_sbs[sg], h)
                    if h == 1:
                        nc.gpsimd.dma_start(
                            out[bh, 1024 * sg : 1024 * (sg + 1), :].rearrange(
                                "(t p) d -> p t d", t=8, p=128
                            ),
                            out_sbs[sg][:],
                        )

                emit_head(0)
                emit_head(1)

                ksT = ps_nd.tile([128, M], F16, tag="fix", bufs=1)
                for j in range(5):
                    nc.tensor.transpose(
                        ksT[:, 128 * j : 128 * (j + 1)],
                        acc_a[:, 128 * j : 128 * (j + 1)],
                        ident_sb[:],
                    )
                with nc.allow_low_precision(reason="bf16 ksum: el err averages out over m"):
                    for j in range(5):
                        nc.vector.reduce_sum(
                            ctx_aug[:, j, 128:129],
                            ksT[:, 128 * j : 128 * (j + 1)],
                            axis=AXX,
                        )
                fixT = ps_nd.tile([128, M], F16, tag="fix", bufs=1)
                for j in range(5):
                    nc.tensor.transpose(
                        fixT[:, 128 * j : 128 * (j + 1)],
                        ctxT_sb[:, 128 * j : 128 * (j + 1)],
                        ident_sb[:],
                    )
                for j in range(5):
                    nc.vector.tensor_copy(
                        ctx_aug[:, j, 0:128], fixT[:, 128 * j : 128 * (j + 1)]
                    )

                for b in range(2, 2 * nsg):
                    emit_tail()
                    emit_head(b)
                emit_tail()
                emit_tail()
    nc.compile()
    return nc


_NC_CACHE = {}


def _get_nc(n_bh=NBH, seq=L):
    key = (n_bh, seq)
    if key not in _NC_CACHE:
        _NC_CACHE[key] = build_bass(n_bh, seq)
    return _NC_CACHE[key]


def host_inputs(projection_matrix):
    projT = np.ascontiguousarray(
        (np.asarray(projection_matrix, dtype=np.float32) / (D**0.25)).T
    ).astype(np.float16)
    ident = np.eye(128, dtype=np.float16)
    return projT, ident


def kernel(q, k, v, projection_matrix, _trace=False, _trace_kwargs=None):
    q = np.asarray(q, dtype=np.float32).reshape(B * H, L, D).astype(np.float16)
    k = np.asarray(k, dtype=np.float32).reshape(B * H, L, D).astype(np.float16)
    v = np.asarray(v, dtype=np.float32).reshape(B * H, L, D).astype(np.float16)
    projT, ident = host_inputs(projection_matrix)

    in_maps = []
    for c in range(NCORES):
        sl = slice(NBH * c, NBH * (c + 1))
        in_maps.append(
            {
                "q": np.ascontiguousarray(q[sl]),
                "k": np.ascontiguousarray(k[sl]),
                "v": np.ascontiguousarray(v[sl]),
                "projT": projT,
                "ident": ident,
            }
        )

    nc = _get_nc()
    kwargs = {}
    if _trace:
        kwargs["trace"] = True
        kwargs.update(_trace_kwargs or {})
    res = run_bass_kernel_spmd(nc, in_maps, core_ids=list(range(NCORES)), **kwargs)
    outs = np.concatenate([res.results[c]["out"] for c in range(NCORES)], axis=0)
    result = outs.reshape(B, H, L, D).astype(np.float32)
    if _trace:
        return result, res
    return result


def timed_run(q, k, v, projection_matrix, iters=5):
    import time
    import jax
    from jax.sharding import Mesh, PartitionSpec
    from jax.experimental.shard_map import shard_map
    from concourse import bass2jax

    q = np.asarray(q, dtype=np.float32).reshape(B * H, L, D).astype(np.float16)
    k = np.asarray(k, dtype=np.float32).reshape(B * H, L, D).astype(np.float16)
    v = np.asarray(v, dtype=np.float32).reshape(B * H, L, D).astype(np.float16)
    projT, ident = host_inputs(projection_matrix)
    nc = _get_nc()
    bass2jax.install_neuronx_cc_hook()

    in_names = []
    out_names = []
    out_avals = []
    zero_outs = []
    import concourse.mybir as mybir_

    partition_name = nc.partition_id_tensor.name if nc.partition_id_tensor else None
    for alloc in nc.m.functions[0].allocations:
        if not isinstance(alloc, mybir_.MemoryLocationSet):
            continue
        name = alloc.memorylocations[0].name
        if alloc.kind == "ExternalInput":
            if name != partition_name:
                in_names.append(name)
        elif alloc.kind == "ExternalOutput":
            out_names.append(name)
            shape = list(alloc.tensor_shape)
            out_avals.append(jax.core.ShapedArray(shape, np.float32))
            zero_outs.append(np.zeros(shape, np.float32))
    n_params = len(in_names)
    n_outs = len(out_names)
    all_names = in_names + out_names
    if partition_name is not None:
        all_names = all_names + [partition_name]

    def _body(*args):
        operands = list(args)
        if partition_name is not None:
            operands.append(bass2jax.partition_id_tensor())
        outs = bass2jax._bass_exec_p.bind(
            *operands,
            out_avals=tuple(out_avals),
            in_names=tuple(all_names),
            out_names=tuple(out_names),
            lowering_input_output_aliases=(),
            sim_require_finite=True,
            sim_require_nnan=True,
            nc=nc,
        )
        return tuple(outs)

    devices = jax.devices()[:NCORES]
    mesh = Mesh(np.asarray(devices), ("core",))
    in_specs = (PartitionSpec("core"),) * (n_params + n_outs)
    out_specs = (PartitionSpec("core"),) * n_outs
    sharded = jax.jit(
        shard_map(_body, mesh=mesh, in_specs=in_specs, out_specs=out_specs, check_rep=False),
        keep_unused=True,
    )

    per_core_vals = {
        "q": [q[NBH * c : NBH * (c + 1)] for c in range(NCORES)],
        "k": [k[NBH * c : NBH * (c + 1)] for c in range(NCORES)],
        "v": [v[NBH * c : NBH * (c + 1)] for c in range(NCORES)],
        "projT": [projT] * NCORES,
        "ident": [ident] * NCORES,
    }
    concat_in = [
        np.concatenate(per_core_vals[nm], axis=0) for nm in in_names
    ]
    concat_zeros = [
        np.zeros((NCORES * z.shape[0], *z.shape[1:]), z.dtype) for z in zero_outs
    ]
    sharding = jax.sharding.NamedSharding(mesh, PartitionSpec("core"))
    dev_in = [jax.device_put(a, sharding) for a in concat_in]
    dev_zero = [jax.device_put(a, sharding) for a in concat_zeros]
    r0 = sharded(*dev_in, *dev_zero)
    jax.block_until_ready(r0)
    times = []
    for _ in range(iters):
        t0 = time.perf_counter()
        rr = sharded(*dev_in, *dev_zero)
        jax.block_until_ready(rr)
        times.append(time.perf_counter() - t0)
    out = np.asarray(rr[out_names.index("out")]).reshape(NCORES, NBH, L, D)
    result = out.reshape(B, H, L, D)
    return result, times


# revision 3
# speedup vs baseline: 414.8200x; 1.0488x over previous
import sys
import math

if "/opt/trn_rl_repo" not in sys.path:
    sys.path.insert(0, "/opt/trn_rl_repo")

import numpy as np
from contextlib import ExitStack

import concourse.bass as bass
import concourse.bacc as bacc
import concourse.mybir as mybir
import concourse.tile as tile
from concourse.bass_utils import run_bass_kernel_spmd

F32 = mybir.dt.float32
F16 = mybir.dt.float16
BF16 = mybir.dt.bfloat16
EXP = mybir.ActivationFunctionType.Exp
MULT = mybir.AluOpType.mult
ADD = mybir.AluOpType.add
AXX = mybir.AxisListType.X

B, H, L, D, M = 8, 4, 4096, 128, 640
NCORES = 8
NBH = (B * H) // NCORES
NEG_GSCALE = -1.0 / (2.0 * math.sqrt(D))


def build_bass(n_bh=NBH, seq=L):
    nc = bacc.Bacc("TRN2", debug=False)
    q = nc.dram_tensor("q", [n_bh, seq, D], F16, kind="ExternalInput").ap()
    k = nc.dram_tensor("k", [n_bh, seq, D], F16, kind="ExternalInput").ap()
    v = nc.dram_tensor("v", [n_bh, seq, D], F16, kind="ExternalInput").ap()
    projT = nc.dram_tensor("projT", [D, M], F16, kind="ExternalInput").ap()
    ident = nc.dram_tensor("ident", [128, 128], F16, kind="ExternalInput").ap()
    out = nc.dram_tensor("out", [n_bh, seq, D], F32, kind="ExternalOutput").ap()

    assert seq % 1024 == 0
    nsg = seq // 1024
    ntile = 8 * nsg

    def ldma(sbuf_tile, dram_ap, sg):
        nc.sync.dma_start(
            sbuf_tile[:],
            dram_ap[1024 * sg : 1024 * (sg + 1), :].rearrange(
                "(t p) d -> p t d", t=8, p=128
            ),
        )

    with tile.TileContext(nc) as tc, ExitStack() as ctx:
        const = ctx.enter_context(tc.tile_pool(name="const", bufs=1))
        warm = const.tile([1, 2], F32)
        nc.vector.memset(warm[:, 0:1], 0.0)
        nc.scalar.activation(warm[:, 1:2], warm[:, 0:1], EXP, bias=0.0, scale=1.0)
        projT_sb = const.tile([D, M], F16)
        ident_sb = const.tile([128, 128], F16)

        ld_k = ctx.enter_context(tc.tile_pool(name="ld_k", bufs=2))
        ld_v = ctx.enter_context(tc.tile_pool(name="ld_v", bufs=2))
        ld_q = ctx.enter_context(tc.tile_pool(name="ld_q", bufs=2))
        kt_p = ctx.enter_context(tc.tile_pool(name="kt_sb", bufs=2))
        qt_p = ctx.enter_context(tc.tile_pool(name="qt_sb", bufs=2))
        phik_p = ctx.enter_context(tc.tile_pool(name="phik", bufs=3))
        phiq_p = ctx.enter_context(tc.tile_pool(name="phiq", bufs=13))
        misc_p = ctx.enter_context(tc.tile_pool(name="misc", bufs=2))
        acc_p = ctx.enter_context(tc.tile_pool(name="acc", bufs=2))
        ctxsb_p = ctx.enter_context(tc.tile_pool(name="ctxsb", bufs=2))
        outsb_p = ctx.enter_context(tc.tile_pool(name="outsb", bufs=2))
        ps_qt = ctx.enter_context(tc.tile_pool(name="ps_qt", bufs=1, space="PSUM"))

        def qt_head(q_buf, h):
            qt_ps = ps_qt.tile([128, 512], F16, tag="qt")
            for t in range(4):
                nc.tensor.transpose(
                    qt_ps[:, 128 * t : 128 * (t + 1)],
                    q_buf[:, 4 * h + t, :],
                    ident_sb[:],
                )
            qt_sb = qt_p.tile([128, 512], F16, tag="qt")
            nc.vector.tensor_copy(qt_sb[:], qt_ps[:])
            return qt_sb

        for bh in range(n_bh):
            acc_a = acc_p.tile([128, M], F16, tag="acc_a")
            acc_b = acc_p.tile([128, M], F16, tag="acc_b")
            ctxT_sb = ctxsb_p.tile([128, M], F16, tag="ctxT")
            q_buf0 = None
            qt_sb0 = None
            with tc.tile_pool(name="ps_ctx", bufs=1, space="PSUM") as ps_ctx, \
                 tc.tile_pool(name="ps_arr", bufs=2, space="PSUM") as ps_arr, \
                 tc.tile_pool(name="ps_kt", bufs=1, space="PSUM") as ps_kt:
                ctxT_ps = ps_ctx.tile([128, M], F32)
                for sg in range(nsg):
                    k_buf = ld_k.tile([128, 8, D], F16, tag="k")
                    if bh == 0 and sg == 0:
                        nc.sync.dma_start(
                            k_buf[:, 0:4, :],
                            k[bh][0:512, :].rearrange("(t p) d -> p t d", t=4, p=128),
                        )
                        nc.sync.dma_start(ident_sb[:], ident)
                        nc.sync.dma_start(
                            k_buf[:, 4:8, :],
                            k[bh][512:1024, :].rearrange("(t p) d -> p t d", t=4, p=128),
                        )
                        nc.sync.dma_start(projT_sb[:], projT)
                    else:
                        ldma(k_buf, k[bh], sg)
                    v_buf = ld_v.tile([128, 8, D], F16, tag="v")
                    ldma(v_buf, v[bh], sg)
                    negb = misc_p.tile([128, 8], F32, tag="negb")
                    gscr = misc_p.tile([128, 8, D], F16, tag="gscr")
                    for u in range(8):
                        nc.vector.tensor_tensor_reduce(
                            out=gscr[:, u, :],
                            in0=k_buf[:, u, :],
                            in1=k_buf[:, u, :],
                            scale=NEG_GSCALE,
                            scalar=0.0,
                            op0=MULT,
                            op1=ADD,
                            accum_out=negb[:, u : u + 1],
                        )
                    for h in range(2):
                        kt_ps = ps_kt.tile([128, 512], F16, tag="kt")
                        for t in range(4):
                            nc.tensor.transpose(
                                kt_ps[:, 128 * t : 128 * (t + 1)],
                                k_buf[:, 4 * h + t, :],
                                ident_sb[:],
                            )
                        kt_sb = kt_p.tile([128, 512], F16, tag="kt")
                        nc.vector.tensor_copy(kt_sb[:], kt_ps[:])
                        if sg == nsg - 1 and h == 1:
                            q_buf0 = ld_q.tile([128, 8, D], F16, tag="q")
                            ldma(q_buf0, q[bh], 0)
                            qt_sb0 = qt_head(q_buf0, 0)
                        for t in range(4):
                            u = 4 * h + t
                            gi = 8 * sg + u
                            arr = ps_arr.tile([128, M], F32, tag="arr")
                            lhsT = kt_sb[:, 128 * t : 128 * (t + 1)]
                            nc.tensor.matmul(arr[:, 0:512], lhsT, projT_sb[:, 0:512])
                            nc.tensor.matmul(arr[:, 512:M], lhsT, projT_sb[:, 512:M])
                            phik = phik_p.tile([128, M], F16, tag="phik")
                            nc.scalar.activation(
                                phik[:], arr[:], EXP, bias=negb[:, u : u + 1], scale=1.0
                            )
                            first = gi == 0
                            last = gi == ntile - 1
                            nc.tensor.matmul(
                                ctxT_ps[:, 0:512], v_buf[:, u, :], phik[:, 0:512],
                                start=first, stop=last,
                            )
                            nc.tensor.matmul(
                                ctxT_ps[:, 512:M], v_buf[:, u, :], phik[:, 512:M],
                                start=first, stop=last,
                            )
                            if gi == 0:
                                nc.vector.tensor_copy(acc_a[:], phik[:])
                            elif gi == 1:
                                nc.gpsimd.tensor_copy(acc_b[:], phik[:])
                            elif gi % 2 == 0:
                                nc.vector.tensor_add(acc_a[:], acc_a[:], phik[:])
                            else:
                                nc.gpsimd.tensor_add(acc_b[:], acc_b[:], phik[:])
                nc.vector.tensor_copy(ctxT_sb[:], ctxT_ps[:])
                nc.gpsimd.tensor_add(acc_a[:], acc_a[:], acc_b[:])

            ctx_aug = ctxsb_p.tile([128, 5, 129], BF16, tag="ctx_aug")
            with tc.tile_pool(name="ps_nd", bufs=2, space="PSUM") as ps_nd, \
                 tc.tile_pool(name="ps_arrq", bufs=2, space="PSUM") as ps_arrq:

                def q_head_rest(qt_sb):
                    phiqs = []
                    for t in range(4):
                        arrq = ps_arrq.tile([128, 5, 128], F32, tag="arrq")
                        rhs = qt_sb[:, 128 * t : 128 * (t + 1)]
                        for j in range(5):
                            nc.tensor.matmul(
                                arrq[:, j, :],
                                projT_sb[:, 128 * j : 128 * (j + 1)],
                                rhs,
                            )
                        phiq = phiq_p.tile([128, 5, 128], BF16, tag="phiq")
                        nc.scalar.activation(phiq[:], arrq[:], EXP, bias=0.0, scale=1.0)
                        phiqs.append(phiq)
                    return phiqs

                def q_tail(phiqs, out_sb, h):
                    for t in range(4):
                        nd = ps_nd.tile([128, 129], F32, tag="nd")
                        for j in range(5):
                            nc.tensor.matmul(
                                nd[:],
                                phiqs[t][:, j, :],
                                ctx_aug[:, j, :],
                                start=(j == 0), stop=(j == 4),
                            )
                        recip = misc_p.tile([128, 1], F32, tag="recip")
                        nc.vector.reciprocal(recip[:], nd[:, 128:129])
                        nc.vector.tensor_scalar_mul(
                            out_sb[:, 4 * h + t, :], nd[:, 0:128], recip[:]
                        )

                q_bufs = {0: q_buf0}
                out_sbs = {}
                pending = []

                def emit_head(b):
                    sg, h = divmod(b, 2)
                    if h == 0:
                        if sg not in q_bufs:
                            q_bufs[sg] = ld_q.tile([128, 8, D], F16, tag="q", name=f"q_buf{sg}")
                            ldma(q_bufs[sg], q[bh], sg)
                        out_sbs[sg] = outsb_p.tile([128, 8, D], F32, tag="out", name=f"out_sb{sg}")
                    qt_sb = qt_sb0 if b == 0 else qt_head(q_bufs[sg], h)
                    pending.append((q_head_rest(qt_sb), sg, h))

                def emit_tail():
                    phiqs, sg, h = pending.pop(0)
                    q_tail(phiqs, out_sbs[sg], h)
                    if h == 1:
                        nc.gpsimd.dma_start(
                            out[bh, 1024 * sg : 1024 * (sg + 1), :].rearrange(
                                "(t p) d -> p t d", t=8, p=128
                            ),
                            out_sbs[sg][:],
                        )

                emit_head(0)
                emit_head(1)

                ksT = ps_nd.tile([128, M], F16, tag="fix", bufs=1)
                for j in range(5):
                    nc.tensor.transpose(
                        ksT[:, 128 * j : 128 * (j + 1)],
                        acc_a[:, 128 * j : 128 * (j + 1)],
                        ident_sb[:],
                    )
                with nc.allow_low_precision(reason="bf16 ksum: el err averages out over m"):
                    for j in range(5):
                        nc.vector.reduce_sum(
                            ctx_aug[:, j, 128:129],
                            ksT[:, 128 * j : 128 * (j + 1)],
                            axis=AXX,
                        )
                fixT = ps_nd.tile([128, M], F16, tag="fix", bufs=1)
                for j in range(5):
                    nc.tensor.transpose(
                        fixT[:, 128 * j : 128 * (j + 1)],
                        ctxT_sb[:, 128 * j : 128 * (j + 1)],
                        ident_sb[:],
                    )
                for j in range(5):
                    nc.vector.tensor_copy(
                        ctx_aug[:, j, 0:128], fixT[:, 128 * j : 128 * (j + 1)]
                    )

                for b in range(2, 2 * nsg):
                    emit_tail()
                    emit_head(b)
                emit_tail()
                emit_tail()
    nc.compile()
    return nc


_NC_CACHE = {}


def _get_nc(n_bh=NBH, seq=L):
    key = (n_bh, seq)
    if key not in _NC_CACHE:
        _NC_CACHE[key] = build_bass(n_bh, seq)
    return _NC_CACHE[key]


def host_inputs(projection_matrix):
    projT = np.ascontiguousarray(
        (np.asarray(projection_matrix, dtype=np.float32) / (D**0.25)).T
    ).astype(np.float16)
    ident = np.eye(128, dtype=np.float16)
    return projT, ident


def kernel(q, k, v, projection_matrix, _trace=False, _trace_kwargs=None):
    q = np.asarray(q, dtype=np.float32).reshape(B * H, L, D).astype(np.float16)
    k = np.asarray(k, dtype=np.float32).reshape(B * H, L, D).astype(np.float16)
    v = np.asarray(v, dtype=np.float32).reshape(B * H, L, D).astype(np.float16)
    projT, ident = host_inputs(projection_matrix)

    in_maps = []
    for c in range(NCORES):
        sl = slice(NBH * c, NBH * (c + 1))
        in_maps.append(
            {
                "q": np.ascontiguousarray(q[sl]),
                "k": np.ascontiguousarray(k[sl]),
                "v": np.ascontiguousarray(v[sl]),
                "projT": projT,
                "ident": ident,
            }
        )

    nc = _get_nc()
    kwargs = {}
    if _trace:
        kwargs["trace"] = True
        kwargs.update(_trace_kwargs or {})
    res = run_bass_kernel_spmd(nc, in_maps, core_ids=list(range(NCORES)), **kwargs)
    outs = np.concatenate([res.results[c]["out"] for c in range(NCORES)], axis=0)
    result = outs.reshape(B, H, L, D).astype(np.float32)
    if _trace:
        return result, res
    return result


def timed_run(q, k, v, projection_matrix, iters=5):
    import time
    import jax
    from jax.sharding import Mesh, PartitionSpec
    from jax.experimental.shard_map import shard_map
    from concourse import bass2jax

    q = np.asarray(q, dtype=np.float32).reshape(B * H, L, D).astype(np.float16)
    k = np.asarray(k, dtype=np.float32).reshape(B * H, L, D).astype(np.float16)
    v = np.asarray(v, dtype=np.float32).reshape(B * H, L, D).astype(np.float16)
    projT, ident = host_inputs(projection_matrix)
    nc = _get_nc()
    bass2jax.install_neuronx_cc_hook()

    in_names = []
    out_names = []
    out_avals = []
    zero_outs = []
    import concourse.mybir as mybir_

    partition_name = nc.partition_id_tensor.name if nc.partition_id_tensor else None
    for alloc in nc.m.functions[0].allocations:
        if not isinstance(alloc, mybir_.MemoryLocationSet):
            continue
        name = alloc.memorylocations[0].name
        if alloc.kind == "ExternalInput":
            if name != partition_name:
                in_names.append(name)
        elif alloc.kind == "ExternalOutput":
            out_names.append(name)
            shape = list(alloc.tensor_shape)
            out_avals.append(jax.core.ShapedArray(shape, np.float32))
            zero_outs.append(np.zeros(shape, np.float32))
    n_params = len(in_names)
    n_outs = len(out_names)
    all_names = in_names + out_names
    if partition_name is not None:
        all_names = all_names + [partition_name]

    def _body(*args):
        operands = list(args)
        if partition_name is not None:
            operands.append(bass2jax.partition_id_tensor())
        outs = bass2jax._bass_exec_p.bind(
            *operands,
            out_avals=tuple(out_avals),
            in_names=tuple(all_names),
            out_names=tuple(out_names),
            lowering_input_output_aliases=(),
            sim_require_finite=True,
            sim_require_nnan=True,
            nc=nc,
        )
        return tuple(outs)

    devices = jax.devices()[:NCORES]
    mesh = Mesh(np.asarray(devices), ("core",))
    in_specs = (PartitionSpec("core"),) * (n_params + n_outs)
    out_specs = (PartitionSpec("core"),) * n_outs
    sharded = jax.jit(
        shard_map(_body, mesh=mesh, in_specs=in_specs, out_specs=out_specs, check_rep=False),
        keep_unused=True,
    )

    per_core_vals = {
        "q": [q[NBH * c : NBH * (c + 1)] for c in range(NCORES)],
        "k": [k[NBH * c : NBH * (c + 1)] for c in range(NCORES)],
        "v": [v[NBH * c : NBH * (c + 1)] for c in range(NCORES)],
        "projT": [projT] * NCORES,
        "ident": [ident] * NCORES,
    }
    concat_in = [
        np.concatenate(per_core_vals[nm], axis=0) for nm in in_names
    ]
    concat_zeros = [
        np.zeros((NCORES * z.shape[0], *z.shape[1:]), z.dtype) for z in zero_outs
    ]
    sharding = jax.sharding.NamedSharding(mesh, PartitionSpec("core"))
    dev_in = [jax.device_put(a, sharding) for a in concat_in]
    dev_zero = [jax.device_put(a, sharding) for a in concat_zeros]
    r0 = sharded(*dev_in, *dev_zero)
    jax.block_until_ready(r0)
    times = []
    for _ in range(iters):
        t0 = time.perf_counter()
        rr = sharded(*dev_in, *dev_zero)
        jax.block_until_ready(rr)
        times.append(time.perf_counter() - t0)
    out = np.asarray(rr[out_names.index("out")]).reshape(NCORES, NBH, L, D)
    result = out.reshape(B, H, L, D)
    return result, times
